# revision 2
# baseline (speedup 1.0000x reference)
"""Trainium2 Bass kernel for DigitConvolutionalModel.

Math: the 3x3 valid conv on the 28x28 image is a linear map, so it folds into
the first Linear layer:
    out = relu(x @ W_eff + b1) @ w2.T + b2
where W_eff[784, 128] = C @ w1.T and C[784, 676] is the conv-as-matrix built
from conv_w.  W_eff is built on the host (O(1) w.r.t. batch); the device does
the two batch matmuls.

Distribution: pure data parallel — batch dim of x sharded across 8 NeuronCores,
weights replicated.  Each core computes out.T [10, 8192]; the host reassembles
[65536, 10].

dtypes: x ships as fp8 e3m4 (4 mantissa bits) — the tolerance is 2e-2 and
e3m4-x against fp16 weights measures 1.3e-2 end-to-end, so this halves HBM
traffic (the binding resource at fp16) and makes the kernel PE-bound.  The PE
takes mixed-dtype operands (stationary W fp16, moving x fp8) at the full
1 col/cycle rate; accumulation is fp32 in PSUM.

Structure: all 16 batch tiles are SBUF-resident (48KB/partition at fp8), and
every x DMA trigger is emitted up-front — tiles 0 and 1 ride solo (one per
HWDGE ring) so the PE starts ~3us after the entry barrier, the rest go in
pairs alternating rings.  Weights ride the gpsimd (SWDGE) ring so they never
delay the x stream.  The epilogue is software-pipelined (tile t's relu/mm2
emitted after tile t+1's mm1 block) and writes into one [10, 8192] SBUF
buffer stored with a single DMA at the end — the baseline's 16 per-tile
stores cost ~14us of serialized trigger+completion tail.
"""

import numpy as np
import ml_dtypes

import concourse.bass as bass  # noqa: F401  (bass registers mybir lowerings)
import concourse.mybir as mybir
import concourse.tile as tile
from concourse import bacc
from concourse.bass_utils import run_bass_kernel_spmd

N_CORES = 8
B = 65536
B_SH = B // N_CORES  # 8192 rows per core
D = 784              # 28*28 input features
DM = 768             # features in the main 128-partition stream
DR = D - DM          # 16 remainder features
H = 128              # hidden
OUT = 10
KT = 128             # contraction tile = full partition dim
NK = DM // KT        # 6 main K-tiles
NB = 512             # batch columns per tile (= one fp32 PSUM bank)
NT = B_SH // NB      # 16 batch tiles
WARMUP = 6           # HAM clock-ramp matmuls before real data arrives

X_DT = "f8"          # "f8" (e3m4) or "f16" for the x stream

_CACHE = {}


def _build_nc():
    f32 = mybir.dt.float32
    f16 = mybir.dt.float16
    fx = mybir.dt.float8e3 if X_DT == "f8" else f16
    nc = bacc.Bacc("TRN2", target_bir_lowering=False, debug=False,
                   num_devices=N_CORES)
    # main x, partition-major: [p, t, k, c] with feature f = k*128 + p
    xtp = nc.dram_tensor("xtp", [KT, NT, NK, NB], fx,
                         kind="ExternalInput").ap()
    # remainder features 768..784: [p, batch] (base partition 0 for all rem
    # matmuls — mixing row-group tile positions reconfigures the PE array)
    xrem = nc.dram_tensor("xrem", [DR, B_SH], fx, kind="ExternalInput").ap()
    weff = nc.dram_tensor("weff", [D, H], f16, kind="ExternalInput").ap()
    w2t = nc.dram_tensor("w2t", [H, OUT], f16, kind="ExternalInput").ap()
    b1c = nc.dram_tensor("b1c", [H, 1], f32, kind="ExternalInput").ap()
    b2c = nc.dram_tensor("b2c", [OUT, 1], f32, kind="ExternalInput").ap()
    out = nc.dram_tensor("out", [OUT, B_SH], f32, kind="ExternalOutput").ap()

    with tile.TileContext(nc) as tc:
        with (
            tc.tile_pool(name="wpool", bufs=1) as wpool,
            tc.tile_pool(name="hpool", bufs=4) as hpool,
            tc.tile_pool(name="ps1", bufs=5, space="PSUM") as ps1pool,
            tc.tile_pool(name="ps2", bufs=2, space="PSUM") as ps2pool,
        ):
            # One SBUF-resident buffer for all 16 x tiles; every trigger is
            # emitted up-front so the two HWDGE rings stream back-to-back
            # with no pool-reuse waits.  Tiles 0/1 go solo (one per ring)
            # so the PE's first real matmul isn't gated on a full pair.
            x_sb = wpool.tile([KT, NT, NK, NB], fx)
            for a, b_ in ((0, 1), (2, 4), (6, 8), (10, 12), (14, 16)):
                nc.sync.dma_start(x_sb[:, a:b_], xtp[:, a:b_])
            for a, b_ in ((1, 2), (4, 6), (8, 10), (12, 14)):
                nc.scalar.dma_start(x_sb[:, a:b_], xtp[:, a:b_])

            # Params ride the gpsimd (SWDGE) ring — ordered by first use so
            # nothing gates the tile-0 matmul chain.
            w_sb = wpool.tile([KT, NK, H], f16)
            nc.gpsimd.dma_start(
                w_sb[:], weff[0:DM, :].rearrange("(k p) m -> p k m", p=KT))
            xr_sb = wpool.tile([DR, B_SH], fx)
            nc.gpsimd.dma_start(xr_sb[:], xrem[:])
            wr_sb = wpool.tile([DR, H], f16)
            nc.gpsimd.dma_start(wr_sb[:], weff[DM:D, :])
            b1_sb = wpool.tile([H, 1], f32)
            nc.gpsimd.dma_start(b1_sb[:], b1c[:])
            w2_sb = wpool.tile([H, OUT], f16)
            nc.gpsimd.dma_start(w2_sb[:], w2t[:])
            b2_sb = wpool.tile([OUT, 1], f32)
            nc.gpsimd.dma_start(b2_sb[:], b2c[:])

            # All 16 output tiles land here; one store at the end.
            o_big = wpool.tile([OUT, B_SH], f32)

            # PE pre-warm: dummy matmuls on a zeroed tile trip the HAM
            # activity monitor toward full clock before real data arrives.
            warm_x = wpool.tile([KT, NB], f16)
            nc.vector.memset(warm_x[:], 0.0)
            warm_ps = ps1pool.tile([H, NB], f32, tag="ps1")
            for _ in range(WARMUP):
                nc.tensor.matmul(warm_ps[:], lhsT=warm_x[:, 0:H],
                                 rhs=warm_x[:], start=True, stop=True)

            def epilogue(t, ps1):
                # h = relu(ps1 + b1), fused on DVE, emitted as fp16
                h_sb = hpool.tile([H, NB], f16)
                nc.vector.tensor_scalar(
                    h_sb[:], ps1[:], b1_sb[:], 0.0,
                    mybir.AluOpType.add, mybir.AluOpType.max)
                # out.T[10, NB] = w2 @ h.T
                ps2 = ps2pool.tile([OUT, NB], f32)
                nc.tensor.matmul(ps2[:], lhsT=w2_sb[:], rhs=h_sb[:],
                                 start=True, stop=True)
                # +b2 on DVE, written straight into the big output buffer
                nc.vector.tensor_scalar_add(
                    o_big[:, t * NB:(t + 1) * NB], ps2[:], b2_sb[:])

            pending = None  # software pipeline: tile t's epilogue is emitted
                            # after tile t+1's mm1 block so PE never waits on
                            # the DVE relu chain
            for t in range(NT):
                # h.T[128, NB] = W_eff.T @ x.T, accumulated over K-tiles.
                ps1 = ps1pool.tile([H, NB], f32)
                for k in range(NK):
                    nc.tensor.matmul(
                        ps1[:],
                        lhsT=w_sb[:, k, :],
                        rhs=x_sb[:, t, k, :],
                        start=(k == 0),
                        stop=False,
                    )
                nc.tensor.matmul(
                    ps1[:], lhsT=wr_sb[:],
                    rhs=xr_sb[:, t * NB:(t + 1) * NB],
                    start=False, stop=True,
                )
                if pending is not None:
                    epilogue(*pending)
                pending = (t, ps1)
            epilogue(*pending)

            # single 320KB store, after every x trigger on its ring
            nc.gpsimd.dma_start(out[:], o_big[:])

    nc.compile()
    return nc


def _get_nc():
    if "nc" not in _CACHE:
        _CACHE["nc"] = _build_nc()
    return _CACHE["nc"]


def _fold_weights(conv_w: np.ndarray, w1: np.ndarray) -> np.ndarray:
    """W_eff[784, 128]: h_pre = x @ W_eff  ==  conv(x) @ w1.T  (float64 accum)."""
    w1k = w1.reshape(H, 26, 26).transpose(1, 2, 0).astype(np.float64)  # [i,j,k]
    cw = conv_w.astype(np.float64)
    W = np.zeros((28, 28, H), np.float64)
    for di in range(3):
        for dj in range(3):
            W[di:di + 26, dj:dj + 26, :] += cw[di, dj] * w1k
    return W.reshape(D, H).astype(np.float32)


def make_in_maps(x, conv_w, w1, b1, w2, b2):
    x = np.asarray(x, np.float32)
    xdt = ml_dtypes.float8_e3m4 if X_DT == "f8" else np.float16
    weff = np.ascontiguousarray(_fold_weights(
        np.asarray(conv_w, np.float32), np.asarray(w1, np.float32))).astype(np.float16)
    w2t = np.ascontiguousarray(np.asarray(w2, np.float32).T).astype(np.float16)
    b1c = np.ascontiguousarray(np.asarray(b1, np.float32).reshape(H, 1))
    b2c = np.ascontiguousarray(np.asarray(b2, np.float32).reshape(OUT, 1))
    in_maps = []
    for i in range(N_CORES):
        xs = x[i * B_SH:(i + 1) * B_SH].astype(xdt)  # [8192, 784]
        # main: [t*NB+c, k*KT+p] -> [p, t, k, c]
        xtp = xs[:, :DM].reshape(NT, NB, NK, KT).transpose(3, 0, 2, 1)
        xrem = xs[:, DM:].T                          # [16, 8192]
        in_maps.append({"xtp": np.ascontiguousarray(xtp),
                        "xrem": np.ascontiguousarray(xrem),
                        "weff": weff, "w2t": w2t, "b1c": b1c, "b2c": b2c})
    return in_maps


def kernel(x, conv_w, w1, b1, w2, b2):
    nc = _get_nc()
    in_maps = make_in_maps(x, conv_w, w1, b1, w2, b2)
    res = run_bass_kernel_spmd(nc, in_maps, list(range(N_CORES)))
    out = np.concatenate([res.results[i]["out"] for i in range(N_CORES)], axis=1)
    return np.ascontiguousarray(out.T)  # [65536, 10] float32


# revision 3
# speedup vs baseline: 1.0967x; 1.0967x over previous
"""Trainium2 Bass kernel for DigitConvolutionalModel.

Math: the 3x3 valid conv on the 28x28 image is a linear map, so it folds into
the first Linear layer:
    out = relu(x @ W_eff + b1) @ w2.T + b2
where W_eff[784, 128] = C @ w1.T and C[784, 676] is the conv-as-matrix built
from conv_w.  W_eff is built on the host (O(1) w.r.t. batch); the device does
the two batch matmuls.

Distribution: pure data parallel — batch dim of x sharded across 8 NeuronCores,
weights replicated.  Each core computes out.T [10, 8192]; the host reassembles
[65536, 10].

dtypes: x ships as fp8 e3m4 (4 mantissa bits) — the tolerance is 2e-2 and
e3m4-x against fp16 weights measures 1.3e-2 end-to-end, so this halves HBM
traffic and makes the kernel PE-bound.  The PE takes mixed-dtype operands
(stationary W fp16, moving x fp8) at the full 1 col/cycle rate; accumulation
is fp32 in PSUM.

PE uniformity: every matmul is a full 128x128xN=512 pass.  The 16 remainder
features (768..784) ride a 7th zero-padded K-tile (zero weight rows x zero
x rows) instead of a K=16 matmul, and w2 is zero-padded to [128, 128] — a
K=16 or M=10 matmul switches the PE row/col group config, which costs ~100ns
on each matmul around the transition (measured: 313ns vs 216ns issue gap).

Schedule: all 16 batch tiles are SBUF-resident (56KB/partition at fp8) and
every DMA trigger is emitted up-front.  sync ring: w[k0] slice, solo tile 0,
then pairs, then the two output half-stores.  scalar ring: w[k1..6], b1,
w2, b2, solo tile 1, then pairs.  The PE's first matmul fires ~3.5us after
the entry barrier; the HAM clock-ramp warmup runs while tile 0 streams.
The epilogue (relu+bias on DVE, mm2, +b2 into one [10, 8192] buffer) is
software-pipelined one tile behind mm1.
"""

import numpy as np
import ml_dtypes

import concourse.bass as bass  # noqa: F401  (bass registers mybir lowerings)
import concourse.mybir as mybir
import concourse.tile as tile
from concourse import bacc
from concourse.bass_utils import run_bass_kernel_spmd

N_CORES = 8
B = 65536
B_SH = B // N_CORES  # 8192 rows per core
D = 784              # 28*28 input features
H = 128              # hidden
OUT = 10
KT = 128             # contraction tile = full partition dim
NK = 7               # K-tiles: 6 full + 1 zero-padded (features 768..784)
NB = 512             # batch columns per tile (= one fp32 PSUM bank)
NT = B_SH // NB      # 16 batch tiles
WARMUP = 6           # HAM clock-ramp matmuls before real data arrives

X_DT = "f8"          # "f8" (e3m4) or "f16" for the x stream

_CACHE = {}


def _build_nc():
    f32 = mybir.dt.float32
    f16 = mybir.dt.float16
    fx = mybir.dt.float8e3 if X_DT == "f8" else f16
    nc = bacc.Bacc("TRN2", target_bir_lowering=False, debug=False,
                   num_devices=N_CORES)
    # main x, partition-major: [p, t, k, c] with feature f = k*128 + p;
    # k=6 carries the 16 remainder features on partitions 0..15, zeros above
    xtp = nc.dram_tensor("xtp", [KT, NT, NK, NB], fx,
                         kind="ExternalInput").ap()
    # weights pre-rearranged on host: wtp[p, k, :] = W_eff[k*128+p, :]
    wtp = nc.dram_tensor("wtp", [KT, NK, H], f16, kind="ExternalInput").ap()
    # w2 zero-padded to M=128 so mm2 keeps the full-array col-group config
    w2p = nc.dram_tensor("w2p", [H, 128], f16, kind="ExternalInput").ap()
    b1c = nc.dram_tensor("b1c", [H, 1], f32, kind="ExternalInput").ap()
    b2c = nc.dram_tensor("b2c", [OUT, 1], f32, kind="ExternalInput").ap()
    out = nc.dram_tensor("out", [OUT, B_SH], f32, kind="ExternalOutput").ap()

    with tile.TileContext(nc) as tc:
        with (
            tc.tile_pool(name="wpool", bufs=1) as wpool,
            tc.tile_pool(name="hpool", bufs=4) as hpool,
            tc.tile_pool(name="ps1", bufs=5, space="PSUM") as ps1pool,
            tc.tile_pool(name="ps2", bufs=2, space="PSUM") as ps2pool,
        ):
            x_sb = wpool.tile([KT, NT, NK, NB], fx)
            w_sb = wpool.tile([KT, NK, H], f16)
            w2_sb = wpool.tile([H, 128], f16)
            b1_sb = wpool.tile([H, 1], f32)
            b2_sb = wpool.tile([OUT, 1], f32)
            o_big = wpool.tile([OUT, B_SH], f32)

            # sync ring: k0 weights (gate of the first matmul), solo tile 0,
            # then pairs.  Ring FIFO order = program order per engine.
            nc.sync.dma_start(w_sb[:, 0:1, :], wtp[:, 0:1, :])
            for a, b_ in ((0, 1), (2, 4), (6, 8), (10, 12), (14, 16)):
                nc.sync.dma_start(x_sb[:, a:b_], xtp[:, a:b_])
            # scalar ring: remaining weights + small params, solo tile 1, pairs
            nc.scalar.dma_start(w_sb[:, 1:NK, :], wtp[:, 1:NK, :])
            nc.scalar.dma_start(b1_sb[:], b1c[:])
            nc.scalar.dma_start(w2_sb[:], w2p[:])
            nc.scalar.dma_start(b2_sb[:], b2c[:])
            for a, b_ in ((1, 2), (4, 6), (8, 10), (12, 14)):
                nc.scalar.dma_start(x_sb[:, a:b_], xtp[:, a:b_])

            # PE pre-warm: dummy matmuls on a zeroed tile trip the HAM
            # activity monitor toward full clock before real data arrives.
            warm_x = wpool.tile([KT, NB], f16)
            nc.vector.memset(warm_x[:], 0.0)
            warm_ps = ps1pool.tile([H, NB], f32, tag="ps1")
            for _ in range(WARMUP):
                nc.tensor.matmul(warm_ps[:], lhsT=warm_x[:, 0:H],
                                 rhs=warm_x[:], start=True, stop=True)

            def epilogue(t, ps1):
                # h = relu(ps1 + b1), fused on DVE, emitted as fp16
                h_sb = hpool.tile([H, NB], f16)
                nc.vector.tensor_scalar(
                    h_sb[:], ps1[:], b1_sb[:], 0.0,
                    mybir.AluOpType.add, mybir.AluOpType.max)
                # out.T[0:10, NB] = w2 @ h.T  (M padded to 128)
                ps2 = ps2pool.tile([128, NB], f32)
                nc.tensor.matmul(ps2[:], lhsT=w2_sb[:], rhs=h_sb[:],
                                 start=True, stop=True)
                # +b2 on DVE, written straight into the big output buffer
                nc.vector.tensor_scalar_add(
                    o_big[:, t * NB:(t + 1) * NB], ps2[0:OUT, :], b2_sb[:])

            pending = None  # software pipeline: tile t's epilogue is emitted
                            # after tile t+1's mm1 block so PE never waits on
                            # the DVE relu chain
            for t in range(NT):
                # h.T[128, NB] = W_eff.T @ x.T, accumulated over 7 K-tiles.
                ps1 = ps1pool.tile([H, NB], f32)
                for k in range(NK):
                    nc.tensor.matmul(
                        ps1[:],
                        lhsT=w_sb[:, k, :],
                        rhs=x_sb[:, t, k, :],
                        start=(k == 0),
                        stop=(k == NK - 1),
                    )
                if pending is not None:
                    epilogue(*pending)
                pending = (t, ps1)
            epilogue(*pending)

            # two half-stores on the sync ring (HWDGE): the first can fire
            # as soon as tiles 0..7 are done, only the second is tail
            half = NT // 2 * NB
            nc.sync.dma_start(out[:, 0:half], o_big[:, 0:half])
            nc.sync.dma_start(out[:, half:B_SH], o_big[:, half:B_SH])

    nc.compile()
    return nc


def _get_nc():
    if "nc" not in _CACHE:
        _CACHE["nc"] = _build_nc()
    return _CACHE["nc"]


def _fold_weights(conv_w: np.ndarray, w1: np.ndarray) -> np.ndarray:
    """W_eff[784, 128]: h_pre = x @ W_eff  ==  conv(x) @ w1.T  (float64 accum)."""
    w1k = w1.reshape(H, 26, 26).transpose(1, 2, 0).astype(np.float64)  # [i,j,k]
    cw = conv_w.astype(np.float64)
    W = np.zeros((28, 28, H), np.float64)
    for di in range(3):
        for dj in range(3):
            W[di:di + 26, dj:dj + 26, :] += cw[di, dj] * w1k
    return W.reshape(D, H).astype(np.float32)


def make_in_maps(x, conv_w, w1, b1, w2, b2):
    x = np.asarray(x, np.float32)
    xdt = ml_dtypes.float8_e3m4 if X_DT == "f8" else np.float16
    weff = _fold_weights(np.asarray(conv_w, np.float32),
                         np.asarray(w1, np.float32))  # [784, 128] f32
    wtp = np.zeros((KT, NK, H), np.float16)
    wtp[:, :6, :] = weff[:768].reshape(6, KT, H).transpose(1, 0, 2)
    wtp[0:16, 6, :] = weff[768:]
    w2p = np.zeros((H, 128), np.float16)
    w2p[:, :OUT] = np.asarray(w2, np.float32).T
    b1c = np.ascontiguousarray(np.asarray(b1, np.float32).reshape(H, 1))
    b2c = np.ascontiguousarray(np.asarray(b2, np.float32).reshape(OUT, 1))
    wtp = np.ascontiguousarray(wtp)
    w2p = np.ascontiguousarray(w2p)
    in_maps = []
    for i in range(N_CORES):
        xs = x[i * B_SH:(i + 1) * B_SH].astype(xdt)  # [8192, 784]
        xtp = np.zeros((KT, NT, NK, NB), xdt)
        # main: [t*NB+c, k*KT+p] -> [p, t, k, c]
        xtp[:, :, :6, :] = xs[:, :768].reshape(NT, NB, 6, KT).transpose(3, 0, 2, 1)
        xtp[0:16, :, 6, :] = xs[:, 768:].reshape(NT, NB, 16).transpose(2, 0, 1)
        in_maps.append({"xtp": np.ascontiguousarray(xtp),
                        "wtp": wtp, "w2p": w2p, "b1c": b1c, "b2c": b2c})
    return in_maps


def kernel(x, conv_w, w1, b1, w2, b2):
    nc = _get_nc()
    in_maps = make_in_maps(x, conv_w, w1, b1, w2, b2)
    res = run_bass_kernel_spmd(nc, in_maps, list(range(N_CORES)))
    out = np.concatenate([res.results[i]["out"] for i in range(N_CORES)], axis=1)
    return np.ascontiguousarray(out.T)  # [65536, 10] float32


# revision 8
# speedup vs baseline: 1.2094x; 1.1028x over previous
"""Trainium2 Bass kernel for DigitConvolutionalModel.

Math: the 3x3 valid conv on the 28x28 image is a linear map, so it folds into
the first Linear layer:
    out = relu(x @ W_eff + b1) @ w2.T + b2
where W_eff[784, 128] = C @ w1.T and C[784, 676] is the conv-as-matrix built
from conv_w.  W_eff is built on the host (O(1) w.r.t. batch); the device does
the two batch matmuls.

Distribution: pure data parallel — batch dim of x sharded across 8 NeuronCores,
weights replicated.  Each core computes out.T [10, 8192]; the host reassembles
[65536, 10].

dtypes: x ships as fp8 e3m4 (4 mantissa bits) — the tolerance is 2e-2 and
e3m4-x against fp16 weights measures 1.3e-2 end-to-end, so this halves HBM
traffic and makes the kernel PE-bound.  The PE takes mixed-dtype operands
(stationary W fp16, moving x fp8) at the full 1 col/cycle rate; accumulation
is fp32 in PSUM.

PE uniformity: every matmul is a full 128x128xN=512 pass.  The 16 remainder
features (768..784) ride a 7th zero-padded K-tile (zero weight rows x zero
x rows) instead of a K=16 matmul, and w2 is zero-padded to [128, 128] — a
K=16 or M=10 matmul switches the PE row/col group config, which costs ~100ns
on each matmul around the transition (measured: 313ns vs 216ns issue gap).

Schedule: all 16 batch tiles are SBUF-resident (56KB/partition at fp8) and
every DMA trigger is emitted up-front.  sync ring: w[k0] slice, solo tile 0,
then pairs, then the two output half-stores.  scalar ring: w[k1..6], b1,
w2, b2, solo tile 1, then pairs.  The PE's first matmul fires ~3.5us after
the entry barrier; the HAM clock-ramp warmup runs while tile 0 streams.
The epilogue (relu+bias on DVE, mm2, +b2 into one [10, 8192] buffer) is
software-pipelined one tile behind mm1.
"""

import numpy as np
import ml_dtypes

import concourse.bass as bass  # noqa: F401  (bass registers mybir lowerings)
import concourse.mybir as mybir
import concourse.tile as tile
from concourse import bacc
from concourse.bass_utils import run_bass_kernel_spmd

N_CORES = 8
B = 65536
B_SH = B // N_CORES  # 8192 rows per core
D = 784              # 28*28 input features
H = 128              # hidden
OUT = 10
KT = 128             # contraction tile = full partition dim
NK = 7               # K-tiles: 6 full + 1 zero-padded (features 768..784)
NB = 512             # batch columns per tile (= one fp32 PSUM bank)
NT = B_SH // NB      # 16 batch tiles
WARMUP = 6           # HAM clock-ramp matmuls before real data arrives

X_DT = "f8"          # "f8" (e3m4) or "f16" for the x stream

_CACHE = {}


def _build_nc():
    f32 = mybir.dt.float32
    f16 = mybir.dt.float16
    fx = mybir.dt.float8e3 if X_DT == "f8" else f16
    nc = bacc.Bacc("TRN2", target_bir_lowering=False, debug=False,
                   num_devices=N_CORES)
    # main x, partition-major: [p, t, k, c] with feature f = k*128 + p;
    # k=6 carries the 16 remainder features on partitions 0..15, zeros above
    xtp = nc.dram_tensor("xtp", [KT, NT, NK, NB], fx,
                         kind="ExternalInput").ap()
    # weights pre-rearranged on host: wtp[p, k, :] = W_eff[k*128+p, :] for
    # k<7; slot 7 is w2 zero-padded to M=128 (full-array col-group for mm2).
    # One contiguous 2KB-per-partition DMA — separate small transfers would
    # be RMW-slow (<512B/partition) and block the ring FIFO.
    wtp = nc.dram_tensor("wtp", [KT, NK + 1, H], f16,
                         kind="ExternalInput").ap()
    b1c = nc.dram_tensor("b1c", [H, 1], f32, kind="ExternalInput").ap()
    b2c = nc.dram_tensor("b2c", [OUT, 1], f32, kind="ExternalInput").ap()
    out = nc.dram_tensor("out", [OUT, B_SH], f32, kind="ExternalOutput").ap()

    with tile.TileContext(nc) as tc:
        with (
            tc.tile_pool(name="wpool", bufs=1) as wpool,
            tc.tile_pool(name="hpool", bufs=4) as hpool,
            tc.tile_pool(name="ps1", bufs=5, space="PSUM") as ps1pool,
            tc.tile_pool(name="ps2", bufs=2, space="PSUM") as ps2pool,
        ):
            x_sb = wpool.tile([KT, NT, NK, NB], fx)
            w_sb = wpool.tile([KT, NK + 1, H], f16)
            b1_sb = wpool.tile([H, 1], f32)
            b2_sb = wpool.tile([OUT, 1], f32)
            o_big = wpool.tile([OUT, B_SH], f32)

            # sync ring: solo tile 0 (gate of the first matmul), then pairs,
            # then the output stores.  Ring FIFO order = program order.
            for a, b_ in ((0, 1), (2, 4), (6, 8), (10, 12), (14, 16)):
                nc.sync.dma_start(x_sb[:, a:b_], xtp[:, a:b_])
            # scalar ring: all weights in one contiguous DMA, solo tile 1,
            # then pairs
            nc.scalar.dma_start(w_sb[:], wtp[:])
            for a, b_ in ((1, 2), (4, 6), (8, 10), (12, 14)):
                nc.scalar.dma_start(x_sb[:, a:b_], xtp[:, a:b_])
            # tiny biases ride SWDGE so their RMW-slow descriptors never sit
            # in front of an x tile; needed only at the first epilogue ~13us
            nc.gpsimd.dma_start(b1_sb[:], b1c[:])
            nc.gpsimd.dma_start(b2_sb[:], b2c[:])

            # PE pre-warm: dummy matmuls on a zeroed tile trip the HAM
            # activity monitor toward full clock before real data arrives.
            warm_x = wpool.tile([KT, NB], f16)
            nc.vector.memset(warm_x[:], 0.0)
            warm_ps = ps1pool.tile([H, NB], f32, tag="ps1")
            for _ in range(WARMUP):
                nc.tensor.matmul(warm_ps[:], lhsT=warm_x[:, 0:H],
                                 rhs=warm_x[:], start=True, stop=True)

            def epilogue(t, ps1):
                # h = relu(ps1 + b1), fused on DVE, emitted as fp16
                h_sb = hpool.tile([H, NB], f16)
                nc.vector.tensor_scalar(
                    h_sb[:], ps1[:], b1_sb[:], 0.0,
                    mybir.AluOpType.add, mybir.AluOpType.max)
                # out.T[0:10, NB] = w2 @ h.T  (M padded to 128)
                ps2 = ps2pool.tile([128, NB], f32)
                nc.tensor.matmul(ps2[:], lhsT=w_sb[:, NK, :], rhs=h_sb[:],
                                 start=True, stop=True)
                # +b2 on DVE, written straight into the big output buffer
                nc.vector.tensor_scalar_add(
                    o_big[:, t * NB:(t + 1) * NB], ps2[0:OUT, :], b2_sb[:])

            pending = None  # software pipeline: tile t's epilogue is emitted
                            # after tile t+1's mm1 block so PE never waits on
                            # the DVE relu chain
            for t in range(NT):
                # h.T[128, NB] = W_eff.T @ x.T, accumulated over 7 K-tiles.
                ps1 = ps1pool.tile([H, NB], f32)
                for k in range(NK):
                    nc.tensor.matmul(
                        ps1[:],
                        lhsT=w_sb[:, k, :],
                        rhs=x_sb[:, t, k, :],
                        start=(k == 0),
                        stop=(k == NK - 1),
                    )
                if pending is not None:
                    epilogue(*pending)
                pending = (t, ps1)
            epilogue(*pending)

            # three stores on the sync ring (HWDGE): the first two fire as
            # their tiles complete mid-loop; only the small last one is tail
            for a, b_ in ((0, 7), (7, 14), (14, 16)):
                nc.sync.dma_start(out[:, a * NB:b_ * NB],
                                  o_big[:, a * NB:b_ * NB])

    nc.compile()
    return nc


def _get_nc():
    if "nc" not in _CACHE:
        _CACHE["nc"] = _build_nc()
    return _CACHE["nc"]


def _fold_weights(conv_w: np.ndarray, w1: np.ndarray) -> np.ndarray:
    """W_eff[784, 128]: h_pre = x @ W_eff  ==  conv(x) @ w1.T  (float64 accum)."""
    w1k = w1.reshape(H, 26, 26).transpose(1, 2, 0).astype(np.float64)  # [i,j,k]
    cw = conv_w.astype(np.float64)
    W = np.zeros((28, 28, H), np.float64)
    for di in range(3):
        for dj in range(3):
            W[di:di + 26, dj:dj + 26, :] += cw[di, dj] * w1k
    return W.reshape(D, H).astype(np.float32)


def make_in_maps(x, conv_w, w1, b1, w2, b2):
    x = np.asarray(x, np.float32)
    xdt = ml_dtypes.float8_e3m4 if X_DT == "f8" else np.float16
    weff = _fold_weights(np.asarray(conv_w, np.float32),
                         np.asarray(w1, np.float32))  # [784, 128] f32
    wtp = np.zeros((KT, NK + 1, H), np.float16)
    wtp[:, :6, :] = weff[:768].reshape(6, KT, H).transpose(1, 0, 2)
    wtp[0:16, 6, :] = weff[768:]
    wtp[:, 7, :OUT] = np.asarray(w2, np.float32).T  # [h, out] block
    b1c = np.ascontiguousarray(np.asarray(b1, np.float32).reshape(H, 1))
    b2c = np.ascontiguousarray(np.asarray(b2, np.float32).reshape(OUT, 1))
    wtp = np.ascontiguousarray(wtp)
    in_maps = []
    for i in range(N_CORES):
        xs = x[i * B_SH:(i + 1) * B_SH].astype(xdt)  # [8192, 784]
        xtp = np.zeros((KT, NT, NK, NB), xdt)
        # main: [t*NB+c, k*KT+p] -> [p, t, k, c]
        xtp[:, :, :6, :] = xs[:, :768].reshape(NT, NB, 6, KT).transpose(3, 0, 2, 1)
        xtp[0:16, :, 6, :] = xs[:, 768:].reshape(NT, NB, 16).transpose(2, 0, 1)
        in_maps.append({"xtp": np.ascontiguousarray(xtp),
                        "wtp": wtp, "b1c": b1c, "b2c": b2c})
    return in_maps


def kernel(x, conv_w, w1, b1, w2, b2):
    nc = _get_nc()
    in_maps = make_in_maps(x, conv_w, w1, b1, w2, b2)
    res = run_bass_kernel_spmd(nc, in_maps, list(range(N_CORES)))
    out = np.concatenate([res.results[i]["out"] for i in range(N_CORES)], axis=1)
    return np.ascontiguousarray(out.T)  # [65536, 10] float32


# revision 12
# speedup vs baseline: 1.2311x; 1.0179x over previous
"""Trainium2 Bass kernel for DigitConvolutionalModel.

Math: the 3x3 valid conv on the 28x28 image is a linear map, so it folds into
the first Linear layer:
    out = relu(x @ W_eff + b1) @ w2.T + b2
where W_eff[784, 128] = C @ w1.T and C[784, 676] is the conv-as-matrix built
from conv_w.  W_eff is built on the host (O(1) w.r.t. batch); the device does
the two batch matmuls.

Distribution: pure data parallel — batch dim of x sharded across 8 NeuronCores,
weights replicated.  Each core computes out.T [10, 8192]; the host reassembles
[65536, 10].

dtypes: x ships as fp8 e3m4 (4 mantissa bits) — the tolerance is 2e-2 and
e3m4-x against fp16 weights measures 1.3e-2 end-to-end, so this halves HBM
traffic and makes the kernel PE-bound.  The PE takes mixed-dtype operands
(stationary W fp16, moving x fp8) at the full 1 col/cycle rate; accumulation
is fp32 in PSUM.

PE uniformity: every matmul is a full 128x128xN=512 pass.  The 16 remainder
features (768..784) ride a 7th zero-padded K-tile (zero weight rows x zero
x rows) instead of a K=16 matmul, and w2 is zero-padded to [128, 128] — a
K=16 or M=10 matmul switches the PE row/col group config, which costs ~100ns
on each matmul around the transition (measured: 313ns vs 216ns issue gap).

Schedule: all 16 batch tiles are SBUF-resident (56KB/partition at fp8) and
every DMA trigger is emitted up-front.  sync ring: w[k0] slice, solo tile 0,
then pairs, then the two output half-stores.  scalar ring: w[k1..6], b1,
w2, b2, solo tile 1, then pairs.  The PE's first matmul fires ~3.5us after
the entry barrier; the HAM clock-ramp warmup runs while tile 0 streams.
The epilogue (relu+bias on DVE, mm2, +b2 into one [10, 8192] buffer) is
software-pipelined one tile behind mm1.
"""

import numpy as np
import ml_dtypes

import concourse.bass as bass  # noqa: F401  (bass registers mybir lowerings)
import concourse.mybir as mybir
import concourse.tile as tile
from concourse import bacc
from concourse.bass_utils import run_bass_kernel_spmd

N_CORES = 8
B = 65536
B_SH = B // N_CORES  # 8192 rows per core
D = 784              # 28*28 input features
H = 128              # hidden
OUT = 10
KT = 128             # contraction tile = full partition dim
NK = 7               # K-tiles: 6 full + 1 zero-padded (features 768..784)
NB = 512             # batch columns per tile (= one fp32 PSUM bank)
NT = B_SH // NB      # 16 batch tiles
WARMUP = 20          # HAM clock-ramp matmuls (N=128) before real data arrives
WARM_N = 128         # small free dim: fine-grained busy bridging, ~107ns each

X_DT = "f8"          # "f8" (e3m4) or "f16" for the x stream

_CACHE = {}


def _build_nc():
    f32 = mybir.dt.float32
    f16 = mybir.dt.float16
    fx = mybir.dt.float8e3 if X_DT == "f8" else f16
    nc = bacc.Bacc("TRN2", target_bir_lowering=False, debug=False,
                   num_devices=N_CORES)
    # main x, partition-major: [p, t, k, c] with feature f = k*128 + p;
    # k=6 carries the 16 remainder features on partitions 0..15, zeros above
    xtp = nc.dram_tensor("xtp", [KT, NT, NK, NB], fx,
                         kind="ExternalInput").ap()
    # weights pre-rearranged on host: wtp[p, k, :] = W_eff[k*128+p, :] for
    # k<7; slot 7 is w2 zero-padded to M=128 (full-array col-group for mm2).
    # One contiguous 2KB-per-partition DMA — separate small transfers would
    # be RMW-slow (<512B/partition) and block the ring FIFO.
    wtp = nc.dram_tensor("wtp", [KT, NK + 1, H], f16,
                         kind="ExternalInput").ap()
    b1c = nc.dram_tensor("b1c", [H, 1], f32, kind="ExternalInput").ap()
    b2c = nc.dram_tensor("b2c", [OUT, 1], f32, kind="ExternalInput").ap()
    out = nc.dram_tensor("out", [OUT, B_SH], f32, kind="ExternalOutput").ap()

    with tile.TileContext(nc) as tc:
        with (
            tc.tile_pool(name="wpool", bufs=1) as wpool,
            tc.tile_pool(name="hpool", bufs=4) as hpool,
            tc.tile_pool(name="ps1", bufs=5, space="PSUM") as ps1pool,
            tc.tile_pool(name="ps2", bufs=2, space="PSUM") as ps2pool,
        ):
            x_sb = wpool.tile([KT, NT, NK, NB], fx)
            w_sb = wpool.tile([KT, NK + 1, H], f16)
            b1_sb = wpool.tile([H, 1], f32)
            b2_sb = wpool.tile([OUT, 1], f32)
            o_big = wpool.tile([OUT, B_SH], f32)

            # sync ring: tile 0 in three k-chunks (the first matmul can start
            # on chunk 1 ~1.5us earlier than on a full tile), then pairs,
            # then the output stores.  Ring FIFO order = program order.
            for ka, kb in ((0, 2), (2, 5), (5, 7)):
                nc.sync.dma_start(x_sb[:, 0:1, ka:kb, :], xtp[:, 0:1, ka:kb, :])
            for a, b_ in ((2, 4), (6, 8), (10, 12)):
                nc.sync.dma_start(x_sb[:, a:b_], xtp[:, a:b_])
            # scalar ring: all weights in one contiguous DMA, solo tile 1,
            # then pairs
            nc.scalar.dma_start(w_sb[:], wtp[:])
            for a, b_ in ((1, 2), (4, 6), (8, 10), (12, 14), (14, 16)):
                nc.scalar.dma_start(x_sb[:, a:b_], xtp[:, a:b_])
            # tiny biases ride SWDGE so their RMW-slow descriptors never sit
            # in front of an x tile; needed only at the first epilogue ~13us
            nc.gpsimd.dma_start(b1_sb[:], b1c[:])
            nc.gpsimd.dma_start(b2_sb[:], b2c[:])

            # PE pre-warm: dummy matmuls on a zeroed tile trip the HAM
            # activity monitor toward full clock before real data arrives.
            warm_x = wpool.tile([KT, NB], f16)
            nc.vector.memset(warm_x[:], 0.0)
            warm_ps = ps1pool.tile([H, NB], f32, tag="ps1")
            for _ in range(WARMUP):
                nc.tensor.matmul(warm_ps[:, 0:WARM_N], lhsT=warm_x[:, 0:H],
                                 rhs=warm_x[:, 0:WARM_N], start=True, stop=True)

            def epilogue(t, ps1):
                # h = relu(ps1 + b1), fused on DVE, emitted as fp16
                h_sb = hpool.tile([H, NB], f16)
                nc.vector.tensor_scalar(
                    h_sb[:], ps1[:], b1_sb[:], 0.0,
                    mybir.AluOpType.add, mybir.AluOpType.max)
                # out.T[0:10, NB] = w2 @ h.T  (M padded to 128)
                ps2 = ps2pool.tile([128, NB], f32)
                nc.tensor.matmul(ps2[:], lhsT=w_sb[:, NK, :], rhs=h_sb[:],
                                 start=True, stop=True)
                # +b2 on DVE, written straight into the big output buffer
                nc.vector.tensor_scalar_add(
                    o_big[:, t * NB:(t + 1) * NB], ps2[0:OUT, :], b2_sb[:])

            pending = None  # software pipeline: tile t's epilogue is emitted
                            # after tile t+1's mm1 block so PE never waits on
                            # the DVE relu chain
            for t in range(NT):
                # h.T[128, NB] = W_eff.T @ x.T, accumulated over 7 K-tiles.
                ps1 = ps1pool.tile([H, NB], f32)
                for k in range(NK):
                    nc.tensor.matmul(
                        ps1[:],
                        lhsT=w_sb[:, k, :],
                        rhs=x_sb[:, t, k, :],
                        start=(k == 0),
                        stop=(k == NK - 1),
                    )
                if pending is not None:
                    epilogue(*pending)
                pending = (t, ps1)
            epilogue(*pending)

            # three stores on the sync ring (HWDGE, idle after its x tiles):
            # the first two fire as their tiles complete mid-loop; only the
            # small last one is tail
            for a, b_ in ((0, 7), (7, 14), (14, 16)):
                nc.sync.dma_start(out[:, a * NB:b_ * NB],
                                  o_big[:, a * NB:b_ * NB])

    nc.compile()
    return nc


def _get_nc():
    if "nc" not in _CACHE:
        _CACHE["nc"] = _build_nc()
    return _CACHE["nc"]


def _fold_weights(conv_w: np.ndarray, w1: np.ndarray) -> np.ndarray:
    """W_eff[784, 128]: h_pre = x @ W_eff  ==  conv(x) @ w1.T  (float64 accum)."""
    w1k = w1.reshape(H, 26, 26).transpose(1, 2, 0).astype(np.float64)  # [i,j,k]
    cw = conv_w.astype(np.float64)
    W = np.zeros((28, 28, H), np.float64)
    for di in range(3):
        for dj in range(3):
            W[di:di + 26, dj:dj + 26, :] += cw[di, dj] * w1k
    return W.reshape(D, H).astype(np.float32)


def make_in_maps(x, conv_w, w1, b1, w2, b2):
    x = np.asarray(x, np.float32)
    xdt = ml_dtypes.float8_e3m4 if X_DT == "f8" else np.float16
    weff = _fold_weights(np.asarray(conv_w, np.float32),
                         np.asarray(w1, np.float32))  # [784, 128] f32
    wtp = np.zeros((KT, NK + 1, H), np.float16)
    wtp[:, :6, :] = weff[:768].reshape(6, KT, H).transpose(1, 0, 2)
    wtp[0:16, 6, :] = weff[768:]
    wtp[:, 7, :OUT] = np.asarray(w2, np.float32).T  # [h, out] block
    b1c = np.ascontiguousarray(np.asarray(b1, np.float32).reshape(H, 1))
    b2c = np.ascontiguousarray(np.asarray(b2, np.float32).reshape(OUT, 1))
    wtp = np.ascontiguousarray(wtp)
    in_maps = []
    for i in range(N_CORES):
        xs = x[i * B_SH:(i + 1) * B_SH].astype(xdt)  # [8192, 784]
        xtp = np.zeros((KT, NT, NK, NB), xdt)
        # main: [t*NB+c, k*KT+p] -> [p, t, k, c]
        xtp[:, :, :6, :] = xs[:, :768].reshape(NT, NB, 6, KT).transpose(3, 0, 2, 1)
        xtp[0:16, :, 6, :] = xs[:, 768:].reshape(NT, NB, 16).transpose(2, 0, 1)
        in_maps.append({"xtp": np.ascontiguousarray(xtp),
                        "wtp": wtp, "b1c": b1c, "b2c": b2c})
    return in_maps


def kernel(x, conv_w, w1, b1, w2, b2):
    nc = _get_nc()
    in_maps = make_in_maps(x, conv_w, w1, b1, w2, b2)
    res = run_bass_kernel_spmd(nc, in_maps, list(range(N_CORES)))
    out = np.concatenate([res.results[i]["out"] for i in range(N_CORES)], axis=1)
    return np.ascontiguousarray(out.T)  # [65536, 10] float32
